# revision 11
# baseline (speedup 1.0000x reference)
"""Trainium2 Bass kernel for nn_AttentionModel (masked single-head attention).

v2: fp8e4m3 DoubleRow matmuls for Q/K projections, scores and attn@V
(2 fp8 MACs/cycle on the PE); V projection stays bf16 for the residual's
accuracy (the +V term dominates the output norm, the attention term is
~0.08x of it, so fp8 noise there is attenuated ~12x).

Math (per batch b):
    Q = X @ Wq + bq ; K = X @ Wk + bk ; V = X @ Wv + bv          X = plms1[b]  [S, D]
    P[s,t] = (Q K^T)[s,t] / sqrt(D),  masked over key t >= L_b
    out = softmax_t(P) @ V + V

Sharding: data-parallel over batch, one NeuronCore per batch (B == 8 cores).

fp8 scaling scheme:
  - host sends Wq' = 16*Wq, Wk' = 16*Wk in fp8 (entries ~N(0, 0.25): in the
    fp8 normal range; unscaled entries ~N(0, 1/1024) would be subnormal).
  - Q' = X8 @ Wq' + 16*bq = 16*Q  (entries ~N(0,256), |.| << 240 fp8 max)
  - scores' = Q'^T K' = 256 * scores; exp scale = (1/sqrt(D))/256 = 1/8192.
  - E = exp(scores - 2 + mask_bias): the -2 shift keeps E in [~0, 32] so
    fp8e4m3 never overflows (TRN fp8 goes Inf above 240); softmax is
    shift-invariant so no max-subtraction is needed (scores are O(1)).
  - mask bias per key position: -2 valid / -30000 masked (exp -> 0).
  - attn@V uses a separate fp8 copy of V (cast from the bf16 V via the PSUM
    epilogue); out = (E@V8)/(E@ones) + V + 2*bv.
"""

import sys

sys.path.insert(0, "/opt/trn_rl_repo")

import numpy as np
import ml_dtypes

import concourse.bass as bass
import concourse.mybir as mybir
import concourse.tile as tile
from concourse.bass_utils import run_bass_kernel_spmd

try:
    import antenv.axon_hooks  # noqa: F401
except ImportError:
    import types

    _hooks = types.ModuleType("antenv.axon_hooks")
    _hooks._hook = None
    _hooks.set_axon_ntff_profile_hook = lambda h: setattr(_hooks, "_hook", h)
    _hooks.get_axon_ntff_profile_hook = lambda: _hooks._hook
    sys.modules["antenv.axon_hooks"] = _hooks

BF16 = mybir.dt.bfloat16
FP8 = mybir.dt.float8e4
F32 = mybir.dt.float32
DR = mybir.MatmulPerfMode.DoubleRow
P = 128
NEG_BIAS = -30000.0
EXP_SHIFT = -2.0
WSCALE = 16.0
N_CORES = 8
FP8NP = ml_dtypes.float8_e4m3


def _split_excess_waits(nc, max_waits=1):
    """Walrus rejects instructions with more than a tiny number of semaphore
    waits; hoist the excess onto same-engine NOPs placed just before."""
    for f in nc.m.functions:
        for bb in f.blocks:
            out = []
            changed = False
            for ins in bb.instructions:
                si = ins.sync_info
                if si is not None and len(si.on_wait) > max_waits:
                    waits = list(si.on_wait)
                    excess, keep = waits[:-max_waits], waits[-max_waits:]
                    for i in range(0, len(excess), max_waits):
                        nop = mybir.InstNoOp(name=f"{ins.name}-wsplit{i}", ins=[], outs=[])
                        nop.engine = ins.engine
                        nop.sync_info = mybir.SyncInfo(
                            on_wait=excess[i : i + max_waits], on_update=[]
                        )
                        nc.register_instruction(nop)
                        out.append(nop)
                    ins.sync_info = mybir.SyncInfo(
                        on_wait=keep, on_update=list(si.on_update)
                    )
                    changed = True
                out.append(ins)
            if changed:
                bb.instructions = out


def build_program(S=2048, DIN=1024, DOUT=1024):
    from contextlib import ExitStack

    KT = DIN // P  # k-tiles over input dim (8)
    KP = KT // 2  # DoubleRow k-pairs (4)
    MT = DOUT // P  # m-tiles over output dim (8)
    TT = S // P  # t-tiles over sequence (16)
    TP = TT // 2  # t-tile pairs for attn@V DR (8)
    NBS = 512  # moving free dim (max for DR: rhs free 2*512=1024)
    SBLK = S // NBS  # s column blocks (4)
    assert S % P == 0 and DIN % 256 == 0 and DOUT % P == 0 and TT % 2 == 0

    nc = bass.Bass("TRN2", target_bir_lowering=False, debug=False)

    xt8a_d = nc.dram_tensor("xt8a", [DIN, 512], FP8, kind="ExternalInput").ap()
    xt8b_d = nc.dram_tensor("xt8b", [DIN, S - 512], FP8, kind="ExternalInput").ap()
    xt_d = nc.dram_tensor("xt", [DIN, S], BF16, kind="ExternalInput").ap()
    wq8_d = nc.dram_tensor("wq8", [DIN, DOUT], FP8, kind="ExternalInput").ap()
    wk8_d = nc.dram_tensor("wk8", [DIN, DOUT], FP8, kind="ExternalInput").ap()
    wv_d = nc.dram_tensor("wv", [DIN, DOUT], BF16, kind="ExternalInput").ap()
    bvb2_d = nc.dram_tensor("bvb2", [P, DOUT], F32, kind="ExternalInput").ap()
    bqt_d = nc.dram_tensor("bqt", [P, MT], F32, kind="ExternalInput").ap()
    bkt_d = nc.dram_tensor("bkt", [P, MT], F32, kind="ExternalInput").ap()
    mkb_d = nc.dram_tensor("mkb", [P, TT], F32, kind="ExternalInput").ap()
    out_d = nc.dram_tensor("out", [S, DOUT], BF16, kind="ExternalOutput").ap()

    # scores' = Q'^T K' = 256 * scores -> exp scale folds in /256
    scl = (1.0 / float(np.sqrt(np.float32(DOUT)))) / (WSCALE * WSCALE)

    with tile.TileContext(nc) as tc, ExitStack() as ctx:
        persist = ctx.enter_context(tc.tile_pool(name="persist", bufs=1))
        qt8 = persist.tile([P, MT, S], FP8)  # Q'^T  [d_out, s]
        kt8 = persist.tile([P, MT, S], FP8)  # K'^T  [d_out, t]
        vv = persist.tile([P, TT, DOUT], BF16)  # V    [t, d]
        v8 = persist.tile([P, TT, DOUT], FP8)  # V fp8 [t, d]
        ones3 = persist.tile([P, 2, 16], FP8)
        bq_sb = persist.tile([P, MT], F32)
        bk_sb = persist.tile([P, MT], F32)
        mk_sb = persist.tile([P, TT], F32)
        bvb2_sb = persist.tile([P, DOUT], F32)

        psum = ctx.enter_context(tc.tile_pool(name="psum", bufs=8, space="PSUM"))

        def acc():
            return psum.tile([P, NBS], F32, name="acc")

        nc.vector.memset(ones3[:], 1.0)

        # PE warmup: burn the cold-HAM window so real matmuls run at 2.4 GHz.
        wrm = persist.tile([P, NBS], BF16, name="warm")
        nc.vector.memset(wrm[:], 0.0)
        wps = psum.tile([P, NBS], F32, name="acc")
        for i in range(8):
            nc.tensor.matmul(
                wps[:], wrm[:, 0:P], wrm[:], start=(i == 0), stop=(i == 7)
            )

        # ---- Phase A: projections ----
        with tc.tile_pool(name="phaseA", bufs=1) as pa:
            xt8a_sb = pa.tile([P, KT, 512], FP8)
            xt8b_sb = pa.tile([P, KT, S - 512], FP8)
            xt_sb = pa.tile([P, KT, S], BF16)
            wq8_sb = pa.tile([P, KT, DOUT], FP8)
            wk8_sb = pa.tile([P, KT, DOUT], FP8)
            wv_sb = pa.tile([P, KT, DOUT], BF16)
            # wq8 + xt8a (first s-block) stream first: Q's first pass can
            # start ~4.5us in, instead of waiting for all 3.1MB of X8.
            for k in range(KT):
                nc.sync.dma_start(wq8_sb[:, k, :], wq8_d[k * P : (k + 1) * P, :])
                nc.sync.dma_start(xt8a_sb[:, k, :], xt8a_d[k * P : (k + 1) * P, :])
            for k in range(KT):
                nc.sync.dma_start(xt8b_sb[:, k, :], xt8b_d[k * P : (k + 1) * P, :])
            nc.sync.dma_start(bq_sb[:], bqt_d[:])
            nc.sync.dma_start(bk_sb[:], bkt_d[:])
            nc.sync.dma_start(mk_sb[:], mkb_d[:])
            nc.sync.dma_start(bvb2_sb[:], bvb2_d[:])
            for k in range(KT):
                nc.sync.dma_start(wk8_sb[:, k, :], wk8_d[k * P : (k + 1) * P, :])
            for k in range(KT):
                nc.sync.dma_start(xt_sb[:, k, :], xt_d[k * P : (k + 1) * P, :])
                nc.sync.dma_start(wv_sb[:, k, :], wv_d[k * P : (k + 1) * P, :])

            acc_i = 0

            def x8mov(kp, sc):
                if sc == 0:
                    return xt8a_sb[:, 2 * kp : 2 * kp + 2, :]
                return xt8b_sb[:, 2 * kp : 2 * kp + 2, (sc - 1) * NBS : sc * NBS]

            def proj_one(w_sb, b_sb, dst, m, scs=None):
                """dst[:, m, :] = fp8(W'[:, m-tile].T @ X8^T + b'), DR pairs.
                One m-tile: 4 PSUM banks (one per s-block) accumulate over
                the 4 DoubleRow k-pairs; stationary reused across s-blocks."""
                nonlocal acc_i
                if scs is None:
                    scs = range(SBLK)
                ps = {sc: acc() for sc in scs}
                for kp in range(KP):
                    for sc in scs:
                        nc.tensor.matmul(
                            ps[sc][:],
                            w_sb[:, 2 * kp : 2 * kp + 2, m * P : (m + 1) * P],
                            x8mov(kp, sc),
                            start=(kp == 0),
                            stop=(kp == KP - 1),
                            perf_mode=DR,
                        )
                for sc in scs:
                    c0 = sc * NBS
                    if acc_i % 2 == 0:
                        nc.scalar.activation(
                            dst[:, m, c0 : c0 + NBS],
                            ps[sc][:],
                            mybir.ActivationFunctionType.Identity,
                            bias=b_sb[:, m : m + 1],
                            scale=1.0,
                        )
                    else:
                        nc.vector.tensor_scalar_add(
                            dst[:, m, c0 : c0 + NBS], ps[sc][:], b_sb[:, m : m + 1]
                        )
                    acc_i += 1

            def v_one(t):
                """vv[:, t, :] (bf16) and v8[:, t, :] (fp8) from one PSUM pass.
                bv is NOT added: attn@(V+bv) + (V+bv) == attn@V + V + 2*bv."""
                pa_, pb_ = acc(), acc()
                for k in range(KT):
                    nc.tensor.matmul(
                        pa_[:],
                        xt_sb[:, k, t * P : (t + 1) * P],
                        wv_sb[:, k, 0:NBS],
                        start=(k == 0),
                        stop=(k == KT - 1),
                    )
                    nc.tensor.matmul(
                        pb_[:],
                        xt_sb[:, k, t * P : (t + 1) * P],
                        wv_sb[:, k, NBS:DOUT],
                        start=(k == 0),
                        stop=(k == KT - 1),
                    )
                nc.vector.tensor_add(vv[:, t, 0:NBS], pa_[:], bvb2_sb[:, 0:NBS])
                nc.scalar.copy(v8[:, t, 0:NBS], pa_[:])
                nc.vector.tensor_add(vv[:, t, NBS:DOUT], pb_[:], bvb2_sb[:, NBS:DOUT])
                nc.scalar.copy(v8[:, t, NBS:DOUT], pb_[:])

            for m in range(MT):
                proj_one(wq8_sb, bq_sb, qt8, m, scs=[0])
            for m in range(MT):
                proj_one(wq8_sb, bq_sb, qt8, m, scs=[1, 2, 3])
            for m in range(MT):
                proj_one(wk8_sb, bk_sb, kt8, m)
            for t in range(TT):
                v_one(t)

        # ---- Phase B: scores' -> masked exp (fp8) -> O = E^T @ V8 ----
        with tc.tile_pool(name="sblk", bufs=1) as sbk:
            for sb in range(SBLK):
                s0 = sb * NBS
                # E[t, s-block] = exp(scl * scores' + mask_bias), fp8
                e_sb = sbk.tile([P, TT, NBS], FP8, name="e", bufs=2)
                for t in range(TT):
                    ps = acc()
                    for kp in range(KP):
                        nc.tensor.matmul(
                            ps[:],
                            kt8[:, 2 * kp : 2 * kp + 2, t * P : (t + 1) * P],
                            qt8[:, 2 * kp : 2 * kp + 2, s0 : s0 + NBS],
                            start=(kp == 0),
                            stop=(kp == KP - 1),
                            perf_mode=DR,
                        )
                    nc.scalar.activation(
                        e_sb[:, t, :],
                        ps[:],
                        mybir.ActivationFunctionType.Exp,
                        bias=mk_sb[:, t : t + 1],
                        scale=scl,
                    )
                for st in range(NBS // P):
                    g = sb * (NBS // P) + st  # global s-tile index
                    r = sbk.tile([P, 1], F32, name="recip", bufs=4)
                    o_sb = sbk.tile([P, DOUT], BF16, name="ostage", bufs=3)
                    po0, po1, pd = acc(), acc(), acc()
                    for tp in range(TP):
                        lhsT = e_sb[:, 2 * tp : 2 * tp + 2, st * P : (st + 1) * P]
                        first, last = tp == 0, tp == TP - 1
                        nc.tensor.matmul(
                            pd[:, 0:1],
                            lhsT,
                            ones3[:, :, 0:1],
                            start=first,
                            stop=last,
                            perf_mode=DR,
                        )
                        nc.tensor.matmul(
                            po0[:],
                            lhsT,
                            v8[:, 2 * tp : 2 * tp + 2, 0:NBS],
                            start=first,
                            stop=last,
                            perf_mode=DR,
                        )
                        nc.tensor.matmul(
                            po1[:],
                            lhsT,
                            v8[:, 2 * tp : 2 * tp + 2, NBS:DOUT],
                            start=first,
                            stop=last,
                            perf_mode=DR,
                        )
                    nc.vector.reciprocal(r[:], pd[:, 0:1])
                    for n, po in ((0, po0), (1, po1)):
                        dsl = slice(n * NBS, (n + 1) * NBS)
                        nc.vector.scalar_tensor_tensor(
                            o_sb[:, dsl],
                            po[:],
                            r[:],
                            vv[:, g, dsl],
                            mybir.AluOpType.mult,
                            mybir.AluOpType.add,
                        )
                        nc.sync.dma_start(out_d[g * P : (g + 1) * P, dsl], o_sb[:, dsl])

    _split_excess_waits(nc)
    return nc


_PROGRAMS = {}


def _get_program(S, DIN, DOUT):
    key = (S, DIN, DOUT)
    if key not in _PROGRAMS:
        _PROGRAMS[key] = build_program(S=S, DIN=DIN, DOUT=DOUT)
    return _PROGRAMS[key]


LAST_RESULTS = None


def _to_fp8(a):
    return np.ascontiguousarray(
        np.clip(np.asarray(a, dtype=np.float32), -240.0, 240.0).astype(FP8NP)
    )


def _host_inputs(plms1, Wq, bq, Wk, bk, Wv, bv, seqlengths, S, DIN, DOUT):
    bf16 = ml_dtypes.bfloat16
    MT = DOUT // P
    TT = S // P
    wq8 = _to_fp8(WSCALE * Wq.astype(np.float32))
    wk8 = _to_fp8(WSCALE * Wk.astype(np.float32))
    wv = np.ascontiguousarray(Wv.astype(bf16))
    bvb2 = np.ascontiguousarray(
        np.broadcast_to((2.0 * bv.astype(np.float32)).reshape(1, DOUT), (P, DOUT))
    )
    bqt = np.ascontiguousarray((WSCALE * bq.astype(np.float32)).reshape(MT, P).T)
    bkt = np.ascontiguousarray((WSCALE * bk.astype(np.float32)).reshape(MT, P).T)
    t_idx = np.arange(S)
    maps = []
    for b in range(plms1.shape[0]):
        xtf = np.ascontiguousarray(plms1[b].T.astype(np.float32))
        xt = np.ascontiguousarray(xtf.astype(bf16))
        xt8 = _to_fp8(xtf)
        xt8a = np.ascontiguousarray(xt8[:, 0:512])
        xt8b = np.ascontiguousarray(xt8[:, 512:])
        L = int(seqlengths[b])
        mkb = np.where(t_idx < L, EXP_SHIFT, NEG_BIAS).astype(np.float32)
        mkb = np.ascontiguousarray(mkb.reshape(TT, P).T)
        maps.append(
            {
                "xt8a": xt8a,
                "xt8b": xt8b,
                "xt": xt,
                "wq8": wq8,
                "wk8": wk8,
                "wv": wv,
                "bvb2": bvb2,
                "bqt": bqt,
                "bkt": bkt,
                "mkb": mkb,
            }
        )
    return maps


def kernel(plms1, Wq, bq, Wk, bk, Wv, bv, seqlengths):
    global LAST_RESULTS
    plms1, Wq, bq, Wk, bk, Wv, bv, seqlengths = (
        np.asarray(a) for a in (plms1, Wq, bq, Wk, bk, Wv, bv, seqlengths)
    )
    B, S, DIN = plms1.shape
    DOUT = Wq.shape[1]
    assert B == N_CORES, f"expected {N_CORES} batches, got {B}"
    nc = _get_program(S, DIN, DOUT)
    in_maps = _host_inputs(plms1, Wq, bq, Wk, bk, Wv, bv, seqlengths, S, DIN, DOUT)
    res = run_bass_kernel_spmd(nc, in_maps, list(range(N_CORES)))
    LAST_RESULTS = res
    out = np.stack([res.results[b]["out"] for b in range(B)]).astype(np.float32)
    return out
